# revision 59
# baseline (speedup 1.0000x reference)
"""Trainium2 Bass kernel for nn_ContrastiveLoss_82300163326281.

Strategy (8 NeuronCores, SPMD, no collectives):
  - Host: L2-normalize the embeddings (f64), transpose to zT [D, B], cast to
    bf16.  Core k receives roll(zT, -1024k, axis=1) so every core runs the
    same program over its local rows 0..1023 (row reductions are permutation
    invariant and the diagonal/positive window is at a fixed local offset).
  - Device, per core (rows = 1024 local rows of sim):
      DMA the bf16 zT panel [2x128, B] into SBUF (chunked, overlapped).
      For each 128-row block x 2048-col chunk:
        PE   : bf16 matmul -> PSUM raw dots v            [128, 2048] f32
        ACT  : E = exp(v*invtemp - c) -> bf16 SBUF, accum -> rowsum(E)
        Pool : scalar_tensor_tensor v*E, accum -> rowsum(v*E)
        DVE  : rowwise min/max of E, skipping the 256-wide diagonal window
               on chunk 0 (contains the diagonal and all K=8 positives);
               E-extremes are v-extremes through the monotone exp.
        ACT+DMA (chunk 0): ship the raw f32 v window [128,256] to DRAM
  - Host finish (exact, f64): per-row min/max merge (log of device E-extremes
    + host window scan), global neg_min/neg_max, affine decomposition of the
    'inverse_sim' weights  w = a*s' + b_r  so that
      sum_j w_j e^{s'_j} = a*sum(s'E) + b_r*sum(E) (+ pos/diag corrections),
    positive log-probs from the shipped windows, weighted mean.

Self-contained: hardcodes shapes; falls back to a pure-numpy replica of the
reference if the positive-index structure is not the expected banded pattern.
"""

import os
import sys

import numpy as np

sys.path.insert(0, "/opt/trn_rl_repo")

B = 8192
D = 256
K = 8
NCORES = 8
ROWS = B // NCORES          # 1024 rows per core
RB = ROWS // 128            # 8 row blocks per core
CHUNK = 2048
NCH = B // CHUNK            # 4 column chunks
WIN = 256                   # diagonal window width (>= 128 + K + 1)
EPS = 1e-8

_state = {}


# --------------------------------------------------------------------------
# device program
# --------------------------------------------------------------------------

def _build_program(invtemp: float, negc: float, stt_engine: str = "dve",
                  gemm: str = "fp8", fd_rbs: tuple = (), fd_delta: float = 0.01):
    from contextlib import ExitStack

    import concourse.bass as bass  # noqa: F401
    import concourse.mybir as mybir
    from concourse import bacc, tile

    f32 = mybir.dt.float32
    bf16 = mybir.dt.bfloat16
    zdt = mybir.dt.float8e4 if gemm == "fp8" else mybir.dt.bfloat16
    AF = mybir.ActivationFunctionType
    ALU = mybir.AluOpType
    AX = mybir.AxisListType

    nc = bacc.Bacc(
        "TRN2",
        target_bir_lowering=False,
        debug=False,
        num_devices=NCORES,
    )
    zt_in = nc.dram_tensor("zt", [2 * 128, B], zdt, kind="ExternalInput").ap()
    # cols 0..RB-1: -it*m_r ; cols RB..2RB-1: (1-delta) * that
    mbias_in = nc.dram_tensor("mbias", [128, 2 * RB], f32, kind="ExternalInput").ap()
    # per rb: [0..3]=sumE per chunk, [4..5]=sumU (chunks 0,2),
    # [6..7]=sumE2 (chunks 1,3); cols 64,65: global E min/max
    stats = nc.dram_tensor("stats", [128, RB * 8 + 2], f32, kind="ExternalOutput").ap()
    wins = nc.dram_tensor("wins", [128, RB * WIN], f32, kind="ExternalOutput").ap()

    with tile.TileContext(nc) as tc, ExitStack() as ctx:
        ztp = ctx.enter_context(tc.tile_pool(name="ztp", bufs=1))
        # zt[:, 0:B] = dims 0..127, zt[:, B:2B] = dims 128..255
        zt = ztp.tile([128, 2 * B], zdt, tag="zt", name="zt")

        psum = ctx.enter_context(tc.tile_pool(name="psum", bufs=2, space="PSUM"))
        Epool = ctx.enter_context(tc.tile_pool(name="Epool", bufs=4))
        runp = ctx.enter_context(tc.tile_pool(name="runp", bufs=1))
        upool = ctx.enter_context(tc.tile_pool(name="upool", bufs=3))
        accp = ctx.enter_context(tc.tile_pool(name="accp", bufs=RB))
        outp = ctx.enter_context(tc.tile_pool(name="outp", bufs=1))
        wsp = ctx.enter_context(tc.tile_pool(name="wsp", bufs=2))

        stats_sb = outp.tile([128, RB * 8 + 2], f32, tag="stats_sb", name="stats_sb")
        nc.gpsimd.memset(stats_sb[:], 0.0)

        # per-row exp biases (host-computed exact diagonal, and scaled copy)
        mbias_sb = outp.tile([128, 2 * RB], f32, tag="mbias_sb", name="mbias_sb")

        # global running elementwise min/max of E across blocks/chunks.  The
        # exp bias is the exact per-row max m_r (the diagonal), so ln of a
        # global E-extreme IS the global extreme of s = v*it - m_r: row/col
        # mixing across blocks loses nothing.
        run_mn = runp.tile([128, CHUNK], bf16, tag="run_mn", name="run_mn")
        run_mx = runp.tile([128, CHUNK], bf16, tag="run_mx", name="run_mx")

        zt_r = zt_in.rearrange("(t p) c -> p t c", p=128)  # [128, 2, B]

        def load_chunk(c):
            for t in range(2):
                nc.sync.dma_start(
                    out=zt[:, t * B + c * CHUNK : t * B + (c + 1) * CHUNK],
                    in_=zt_r[:, t, c * CHUNK : (c + 1) * CHUNK],
                )

        zt_v = zt[:].rearrange("p (t c) -> p t c", t=2)  # [128, 2, B]

        def main_block(rb, c):  # noqa: C901
            pt = psum.tile([128, CHUNK], f32, tag="pt", name=f"pt{rb}_{c}")
            if gemm == "fp8":
                # DoubleRow: both 128-dim k-tiles in one matmul
                lw = zt_v[:, :, 128 * rb : 128 * rb + 128]  # [128, 2, 128]
                for b in range(CHUNK // 512):
                    col = CHUNK * c + 512 * b
                    nc.tensor.matmul(
                        pt[:, 512 * b : 512 * b + 512],
                        lhsT=lw,
                        rhs=zt_v[:, :, col : col + 512],
                        perf_mode=mybir.MatmulPerfMode.DoubleRow,
                        start=True,
                        stop=True,
                    )
            else:
                l0 = zt[:, 128 * rb : 128 * rb + 128]
                l1 = zt[:, B + 128 * rb : B + 128 * rb + 128]
                for b in range(CHUNK // 512):
                    col = CHUNK * c + 512 * b
                    nc.tensor.matmul(
                        pt[:, 512 * b : 512 * b + 512],
                        lhsT=l0,
                        rhs=zt[:, col : col + 512],
                        start=True,
                        stop=False,
                    )
                for b in range(CHUNK // 512):
                    col = CHUNK * c + 512 * b
                    nc.tensor.matmul(
                        pt[:, 512 * b : 512 * b + 512],
                        lhsT=l1,
                        rhs=zt[:, B + col : B + col + 512],
                        start=False,
                        stop=True,
                    )

            if c == 0:
                # ship the raw f32 v window (positives + diagonal)
                o = 128 * rb
                wstage = wsp.tile([128, WIN], f32, tag="wstage", name=f"ws{rb}")
                nc.scalar.copy(wstage[:], pt[:, o : o + WIN])
                nc.sync.dma_start(
                    out=wins[:, WIN * rb : WIN * rb + WIN],
                    in_=wstage[:],
                )

            E = Epool.tile([128, CHUNK], bf16, tag="E", name=f"E{rb}_{c}")
            nc.scalar.activation(
                E[:],
                pt[:],
                AF.Exp,
                bias=mbias_sb[:, rb : rb + 1],
                scale=float(invtemp),
                accum_out=stats_sb[:, 8 * rb + c : 8 * rb + c + 1],
            )
            u = upool.tile([128, CHUNK], bf16, tag="u", name=f"u{rb}_{c}")
            if c % 2 == 1:
                # odd chunks: second exp at scale (1-delta) — the finite
                # difference in exponent scale yields sum(s*e^s) on the host
                # (runs on ACT instead of loading DVE with an STT)
                nc.scalar.activation(
                    u[:],
                    pt[:],
                    AF.Exp,
                    bias=mbias_sb[:, RB + rb : RB + rb + 1],
                    scale=float(np.float32(invtemp * (1.0 - fd_delta))),
                    accum_out=stats_sb[:, 8 * rb + 6 + c // 2 : 8 * rb + 7 + c // 2],
                )
            else:
                stt = nc.gpsimd if stt_engine == "pool" else nc.vector
                stt.scalar_tensor_tensor(
                    out=u[:],
                    in0=pt[:],
                    scalar=1.0,
                    in1=E[:],
                    op0=ALU.bypass,
                    op1=ALU.mult,
                    accum_out=stats_sb[:, 8 * rb + 4 + c // 2 : 8 * rb + 5 + c // 2],
                )
            if c == 0:
                # sanitize the diag/positive window with neighbouring
                # (negative) columns so extremes can run on whole tiles
                o = 128 * rb
                nc.scalar.copy(E[:, o : o + WIN], E[:, o + WIN : o + 2 * WIN])
            # inline running min/max update (2x DVE mode on bf16)
            if _state["first_E"] is None and not _state["run_init"]:
                _state["first_E"] = E
            elif _state["run_init"]:
                nc.vector.tensor_tensor(run_mn[:], run_mn[:], E[:], op=ALU.min)
                nc.vector.tensor_tensor(run_mx[:], run_mx[:], E[:], op=ALU.max)
            else:
                first = _state["first_E"]
                nc.vector.tensor_tensor(run_mn[:], first[:], E[:], op=ALU.min)
                nc.vector.tensor_tensor(run_mx[:], first[:], E[:], op=ALU.max)
                _state["first_E"] = None
                _state["run_init"] = True

        _state["first_E"] = None
        _state["run_init"] = False

        load_chunk(0)
        nc.sync.dma_start(out=mbias_sb[:], in_=mbias_in)
        for c in range(1, NCH):
            load_chunk(c)
        for rb in range(RB):
            for c in range(NCH):
                main_block(rb, c)

        G = RB * 8
        nc.vector.tensor_reduce(
            stats_sb[:, G : G + 1], run_mn[:], axis=AX.X, op=ALU.min
        )
        nc.vector.tensor_reduce(
            stats_sb[:, G + 1 : G + 2], run_mx[:], axis=AX.X, op=ALU.max
        )

        nc.sync.dma_start(out=stats, in_=stats_sb[:])

        _state.pop("first_E", None)
        _state.pop("run_init", None)

        _state.pop("acc", None)
        _state.pop("pidx", None)

    nc.compile()
    return nc


# --------------------------------------------------------------------------
# runners
# --------------------------------------------------------------------------

FD_DELTA = 0.01


def _fd_rbs():
    s = os.environ.get("KERNEL_FD_RBS", "0,2,4,6")
    return tuple(int(x) for x in s.split(",") if x != "")


def _get_program(invtemp: float, negc: float):
    stt_engine = os.environ.get("KERNEL_STT_ENGINE", "dve")
    gemm = os.environ.get("KERNEL_GEMM", "fp8")
    fd_rbs = _fd_rbs()
    key = ("prog", float(invtemp), float(negc), stt_engine, gemm, fd_rbs)
    if key not in _state:
        _state[key] = _build_program(invtemp, negc, stt_engine, gemm,
                                     fd_rbs, FD_DELTA)
    return _state[key]


def _run_device_stock(nc, in_maps):
    from concourse.bass_utils import run_bass_kernel_spmd

    res = run_bass_kernel_spmd(nc, in_maps, list(range(NCORES)))
    _state["last_results"] = res
    return res.results


def _make_cached_runner(nc, return_parts=False):
    """Vendored multi-core tail of bass2jax.run_bass_via_pjrt, but keeping the
    jitted callable so repeated invocations (for timing) do not recompile."""
    import jax
    import concourse.mybir as mybir
    from jax.sharding import Mesh, PartitionSpec
    from concourse.bass2jax import (
        _bass_exec_p,
        install_neuronx_cc_hook,
        partition_id_tensor,
    )

    try:
        from jax.experimental.shard_map import shard_map
    except Exception:  # newer jax
        from jax import shard_map  # type: ignore

    install_neuronx_cc_hook()

    partition_name = nc.partition_id_tensor.name if nc.partition_id_tensor else None
    in_names, out_names, out_avals, zero_outs = [], [], [], []
    for alloc in nc.m.functions[0].allocations:
        if not isinstance(alloc, mybir.MemoryLocationSet):
            continue
        name = alloc.memorylocations[0].name
        if alloc.kind == "ExternalInput":
            if name != partition_name:
                in_names.append(name)
        elif alloc.kind == "ExternalOutput":
            out_names.append(name)
            shape = tuple(alloc.tensor_shape)
            dtype = mybir.dt.np(alloc.dtype)
            out_avals.append(jax.core.ShapedArray(shape, dtype))
            zero_outs.append(np.zeros(shape, dtype))
    n_params = len(in_names)
    all_names = in_names + out_names
    if partition_name is not None:
        all_names = all_names + [partition_name]
    donate = tuple(range(n_params, n_params + len(out_names)))

    def _body(*args):
        operands = list(args)
        if partition_name is not None:
            operands.append(partition_id_tensor())
        outs = _bass_exec_p.bind(
            *operands,
            out_avals=tuple(out_avals),
            in_names=tuple(all_names),
            out_names=tuple(out_names),
            lowering_input_output_aliases=(),
            sim_require_finite=True,
            sim_require_nnan=True,
            nc=nc,
        )
        return tuple(outs)

    devices = jax.devices()[:NCORES]
    mesh = Mesh(np.asarray(devices), ("core",))
    n_out = len(out_names)
    sharded = jax.jit(
        shard_map(
            _body,
            mesh=mesh,
            in_specs=(PartitionSpec("core"),) * (n_params + n_out),
            out_specs=(PartitionSpec("core"),) * n_out,
            check_rep=False,
        ),
        donate_argnums=donate,
        keep_unused=True,
    )

    def run(in_maps):
        concat_in = [
            np.concatenate([np.asarray(m[nm]) for m in in_maps], axis=0)
            for nm in in_names
        ]
        concat_zeros = [
            np.zeros((NCORES * z.shape[0], *z.shape[1:]), z.dtype) for z in zero_outs
        ]
        out_arrs = sharded(*concat_in, *concat_zeros)
        return [
            {
                nm: np.asarray(out_arrs[i]).reshape(NCORES, *out_avals[i].shape)[c]
                for i, nm in enumerate(out_names)
            }
            for c in range(NCORES)
        ]

    if return_parts:
        return run, sharded, in_names, out_avals, zero_outs
    return run


def _run_device(nc, in_maps):
    if os.environ.get("KERNEL_FAST_RUNNER"):
        key = ("runner", id(nc))
        if key not in _state:
            _state[key] = _make_cached_runner(nc)
        return _state[key](in_maps)
    return _run_device_stock(nc, in_maps)


# --------------------------------------------------------------------------
# host finish
# --------------------------------------------------------------------------

def _numpy_reference(emb, pos_vals, temperature, pos_row, pos_col):
    """Exact fallback replica of the reference (used only if the positive
    index pattern is not the expected banded structure)."""
    n = emb.shape[0]
    norm = np.sqrt((emb.astype(np.float32) ** 2).sum(1, keepdims=True))
    z = emb / np.maximum(norm, np.float32(1e-12))
    temp = np.float32(np.log1p(np.exp(np.float64(temperature))))
    sim = (z @ z.T) / temp
    sim = sim - sim.max(axis=1, keepdims=True)
    posd = np.zeros((n, n), bool)
    posd[pos_row, pos_col] = True
    negm = ~posd & ~np.eye(n, dtype=bool)
    pos_w = 1.0 - pos_vals
    pos_w = (pos_w - pos_w.min()) / (pos_w.max() - pos_w.min() + np.float32(EPS))
    neg_min = sim[negm].min()
    neg_max = sim[negm].max()
    neg_w = (sim - neg_min) / (neg_max - neg_min + np.float32(EPS)) + 1.0
    logw = np.where(negm, np.log(neg_w), 0.0).astype(np.float32)
    a = (sim + logw).astype(np.float64)
    lse = np.log(np.exp(a).sum(1))
    pl = sim[pos_row, pos_col].astype(np.float64) - lse[pos_row]
    return np.float32(-np.mean(pl * pos_w.astype(np.float64)))


def _host_prepare(emb):
    """Normalize (f64), transpose, cast to the GEMM dtype: zT [D, B].
    Also return the per-row squared norm of the quantized z (the exact sim
    diagonal = per-row max, used as the exp shift)."""
    import ml_dtypes

    gemm = os.environ.get("KERNEL_GEMM", "fp8")
    zdt = ml_dtypes.float8_e4m3 if gemm == "fp8" else ml_dtypes.bfloat16
    e = emb.astype(np.float64)
    nrm = np.sqrt((e * e).sum(1, keepdims=True))
    z = e / np.maximum(nrm, 1e-12)
    zT = np.ascontiguousarray(z.T.astype(np.float32)).astype(zdt)
    zq = zT.astype(np.float32)
    diag_q = (zq * zq).sum(0).astype(np.float32)  # [B]
    return zT, diag_q  # [256, 8192], [8192]


def _make_in_maps(emb, invtemp):
    zT, diag_q = _host_prepare(emb)
    it2 = np.float32(np.float32(invtemp) * np.float32(1.0 - FD_DELTA))
    lam = np.float64(it2) / np.float64(np.float32(invtemp))
    in_maps = []
    for k in range(NCORES):
        dk = np.roll(diag_q, -ROWS * k)[0:ROWS]  # local rows of core k
        mb = np.ascontiguousarray(
            (-np.float32(invtemp) * dk).reshape(RB, 128).T
        ).astype(np.float32)                     # [128, RB]
        mb2 = (mb.astype(np.float64) * lam).astype(np.float32)
        in_maps.append(
            {
                "zt": np.roll(zT, -ROWS * k, axis=1),
                "mbias": np.concatenate([mb, mb2], axis=1),
            }
        )
    return in_maps, diag_q


def kernel(**inputs):
    emb = np.ascontiguousarray(np.asarray(inputs["embeddings"], dtype=np.float32))
    pos_vals = np.asarray(inputs["pos_vals"], dtype=np.float32)
    temperature = np.asarray(inputs["temperature"], dtype=np.float32)
    pos_row = np.asarray(inputs["pos_row"]).astype(np.int64)
    pos_col = np.asarray(inputs["pos_col"]).astype(np.int64)

    rr = np.repeat(np.arange(B, dtype=np.int64), K)
    oo = np.tile(np.arange(1, K + 1, dtype=np.int64), B)
    structured = (
        emb.shape == (B, D)
        and pos_row.shape == (B * K,)
        and np.array_equal(pos_row, rr)
        and np.array_equal(pos_col, (rr + oo) % B)
    )
    if not structured:
        return _numpy_reference(emb, pos_vals, temperature, pos_row, pos_col)

    temp = float(np.log1p(np.exp(np.float64(temperature))))
    invtemp = 1.0 / np.float32(temp)  # f32 to match device immediates
    invtemp = float(np.float32(invtemp))
    c = invtemp  # row max == diagonal == 1/temp (up to bf16 noise; c is a
    # shift constant only, the host uses the exact shipped diagonal)
    negc = float(np.float32(-c))

    nc = _get_program(invtemp, negc)
    in_maps, diag_q = _make_in_maps(emb, invtemp)
    results = _run_device(nc, in_maps)

    # ---- host finish (f64) ----
    it = np.float64(invtemp)
    cc = np.float64(c)

    sumE = np.empty(B)
    sumU = np.empty(B)
    sumE2 = np.empty(B)
    se_even = np.empty(B)
    se_odd = np.empty(B)
    Wv = np.empty((B, WIN))
    emin_glob = np.inf
    emax_glob = -np.inf

    # the device exp shift: s = v*it - m_r with m_r = it * diag_q
    m = diag_q.astype(np.float64) * it

    ridx = np.arange(128)
    G = RB * 8
    for k in range(NCORES):
        stats = results[k]["stats"].astype(np.float64)  # [128, RB*8+2]
        wins = results[k]["wins"].astype(np.float64)    # [128, RB*WIN]
        emin_glob = min(emin_glob, stats[:, G].min())
        emax_glob = max(emax_glob, stats[:, G + 1].max())
        for rb in range(RB):
            g0 = ROWS * k + 128 * rb
            s = stats[:, 8 * rb : 8 * rb + 8]
            W = wins[:, WIN * rb : WIN * rb + WIN]  # [128, 256] raw v
            sumE[g0 : g0 + 128] = s[:, 0:4].sum(1)
            se_even[g0 : g0 + 128] = s[:, 0] + s[:, 2]
            se_odd[g0 : g0 + 128] = s[:, 1] + s[:, 3]
            sumU[g0 : g0 + 128] = s[:, 4] + s[:, 5]
            sumE2[g0 : g0 + 128] = s[:, 6] + s[:, 7]
            Wv[g0 : g0 + 128] = W

    # masked min/max inside window: exclude relative cols r..r+K
    wmin = np.empty(B)
    wmax = np.empty(B)
    for blk in range(B // 128):
        sl = slice(blk * 128, blk * 128 + 128)
        Wm = Wv[sl].copy()
        for o in range(K + 1):
            Wm[ridx, ridx + o] = np.nan
        wmin[sl] = np.nanmin(Wm, axis=1)
        wmax[sl] = np.nanmax(Wm, axis=1)

    # global neg extremes of s = v*it - m_r.  The device exp bias is the
    # exact per-row m_r, so ln(E-extreme) IS the s-extreme.
    neg_min = min(np.log(emin_glob), ((wmin * it) - m).min())
    neg_max = max(np.log(emax_glob), ((wmax * it) - m).max())
    a = 1.0 / (neg_max - neg_min + EPS)
    b = 1.0 - a * neg_min

    # pos/diag (pd) corrections from the raw windows
    rows = np.arange(B)
    r_in_blk = rows % 128
    pd_idx = r_in_blk[:, None] + np.arange(K + 1)[None, :]   # [B, 9] window cols
    v_pd = Wv[rows[:, None], pd_idx]                         # raw v at diag+pos
    s_pd = v_pd * it - m[:, None]                            # s = v*it - m_r
    E_pd = np.exp(s_pd)
    sum_pd_E = E_pd.sum(1)
    sum_pd_sE = (s_pd * E_pd).sum(1)

    # sum sE over all cols: exact (it*sumU - m*sumE) on even chunks (STT),
    # finite difference (se_odd - sumE2)/delta_eff on odd chunks
    it2 = np.float32(np.float32(invtemp) * np.float32(1.0 - FD_DELTA))
    delta_eff = 1.0 - np.float64(it2) / np.float64(np.float32(invtemp))
    A_all = (it * sumU - m * se_even) + (se_odd - sumE2) / delta_eff
    A_neg = A_all - sum_pd_sE
    B_neg = sumE - sum_pd_E

    Sw = a * A_neg + b * B_neg + sum_pd_E
    log_sw = np.log(Sw)

    # positive log-probs: pos o (o=1..K) of row r is window col r_in_blk+o
    v_pos = v_pd[:, 1:]                      # [B, K]
    pos_log = v_pos * it - m[:, None] - log_sw[:, None]

    pos_w = 1.0 - pos_vals.astype(np.float64)
    pos_w = (pos_w - pos_w.min()) / (pos_w.max() - pos_w.min() + EPS)
    loss = -np.mean(pos_log.reshape(-1) * pos_w)
    return np.float32(loss)


# revision 60
# speedup vs baseline: 1.1623x; 1.1623x over previous
"""Trainium2 Bass kernel for nn_ContrastiveLoss_82300163326281.

Strategy (8 NeuronCores, SPMD, no collectives):
  - Host: L2-normalize the embeddings (f64), transpose to zT [D, B], cast to
    bf16.  Core k receives roll(zT, -1024k, axis=1) so every core runs the
    same program over its local rows 0..1023 (row reductions are permutation
    invariant and the diagonal/positive window is at a fixed local offset).
  - Device, per core (rows = 1024 local rows of sim):
      DMA the bf16 zT panel [2x128, B] into SBUF (chunked, overlapped).
      For each 128-row block x 2048-col chunk:
        PE   : bf16 matmul -> PSUM raw dots v            [128, 2048] f32
        ACT  : E = exp(v*invtemp - c) -> bf16 SBUF, accum -> rowsum(E)
        Pool : scalar_tensor_tensor v*E, accum -> rowsum(v*E)
        DVE  : rowwise min/max of E, skipping the 256-wide diagonal window
               on chunk 0 (contains the diagonal and all K=8 positives);
               E-extremes are v-extremes through the monotone exp.
        ACT+DMA (chunk 0): ship the raw f32 v window [128,256] to DRAM
  - Host finish (exact, f64): per-row min/max merge (log of device E-extremes
    + host window scan), global neg_min/neg_max, affine decomposition of the
    'inverse_sim' weights  w = a*s' + b_r  so that
      sum_j w_j e^{s'_j} = a*sum(s'E) + b_r*sum(E) (+ pos/diag corrections),
    positive log-probs from the shipped windows, weighted mean.

Self-contained: hardcodes shapes; falls back to a pure-numpy replica of the
reference if the positive-index structure is not the expected banded pattern.
"""

import os
import sys

import numpy as np

sys.path.insert(0, "/opt/trn_rl_repo")

B = 8192
D = 256
K = 8
NCORES = 8
ROWS = B // NCORES          # 1024 rows per core
RB = ROWS // 128            # 8 row blocks per core
CHUNK = 2048
NCH = B // CHUNK            # 4 column chunks
WIN = 256                   # diagonal window width (>= 128 + K + 1)
EPS = 1e-8

_state = {}


# --------------------------------------------------------------------------
# device program
# --------------------------------------------------------------------------

def _build_program(invtemp: float, negc: float, stt_engine: str = "dve",
                  gemm: str = "fp8", fd_rbs: tuple = (), fd_delta: float = 0.01):
    from contextlib import ExitStack

    import concourse.bass as bass  # noqa: F401
    import concourse.mybir as mybir
    from concourse import bacc, tile

    f32 = mybir.dt.float32
    bf16 = mybir.dt.bfloat16
    zdt = mybir.dt.float8e4 if gemm == "fp8" else mybir.dt.bfloat16
    AF = mybir.ActivationFunctionType
    ALU = mybir.AluOpType
    AX = mybir.AxisListType

    nc = bacc.Bacc(
        "TRN2",
        target_bir_lowering=False,
        debug=False,
        num_devices=NCORES,
    )
    zt_in = nc.dram_tensor("zt", [2 * 128, B], zdt, kind="ExternalInput").ap()
    # cols 0..RB-1: -it*m_r ; cols RB..2RB-1: (1-delta) * that
    mbias_in = nc.dram_tensor("mbias", [128, 2 * RB], f32, kind="ExternalInput").ap()
    # per rb: [0..3]=sumE per chunk, [4..5]=sumU (chunks 0,2),
    # [6..7]=sumE2 (chunks 1,3); cols 64,65: global E min/max
    stats = nc.dram_tensor("stats", [128, RB * 8 + 2], f32, kind="ExternalOutput").ap()
    wins = nc.dram_tensor("wins", [128, RB * WIN], f32, kind="ExternalOutput").ap()

    with tile.TileContext(nc) as tc, ExitStack() as ctx:
        ztp = ctx.enter_context(tc.tile_pool(name="ztp", bufs=1))
        # zt[:, 0:B] = dims 0..127, zt[:, B:2B] = dims 128..255
        zt = ztp.tile([128, 2 * B], zdt, tag="zt", name="zt")

        psum = ctx.enter_context(tc.tile_pool(name="psum", bufs=2, space="PSUM"))
        Epool = ctx.enter_context(tc.tile_pool(name="Epool", bufs=3))
        runp = ctx.enter_context(tc.tile_pool(name="runp", bufs=1))
        upool = ctx.enter_context(tc.tile_pool(name="upool", bufs=2))
        accp = ctx.enter_context(tc.tile_pool(name="accp", bufs=RB))
        outp = ctx.enter_context(tc.tile_pool(name="outp", bufs=1))
        wsp = ctx.enter_context(tc.tile_pool(name="wsp", bufs=2))

        stats_sb = outp.tile([128, RB * 8 + 2], f32, tag="stats_sb", name="stats_sb")
        nc.gpsimd.memset(stats_sb[:], 0.0)

        # per-row exp biases (host-computed exact diagonal, and scaled copy)
        mbias_sb = outp.tile([128, 2 * RB], f32, tag="mbias_sb", name="mbias_sb")

        # global running elementwise min/max of E across blocks/chunks.  The
        # exp bias is the exact per-row max m_r (the diagonal), so ln of a
        # global E-extreme IS the global extreme of s = v*it - m_r: row/col
        # mixing across blocks loses nothing.
        run_mn = runp.tile([128, CHUNK], bf16, tag="run_mn", name="run_mn")
        run_mx = runp.tile([128, CHUNK], bf16, tag="run_mx", name="run_mx")

        zt_r = zt_in.rearrange("(t p) c -> p t c", p=128)  # [128, 2, B]

        def load_chunk(c):
            for t in range(2):
                nc.sync.dma_start(
                    out=zt[:, t * B + c * CHUNK : t * B + (c + 1) * CHUNK],
                    in_=zt_r[:, t, c * CHUNK : (c + 1) * CHUNK],
                )

        zt_v = zt[:].rearrange("p (t c) -> p t c", t=2)  # [128, 2, B]

        def main_block(rb, c):  # noqa: C901
            pt = psum.tile([128, CHUNK], f32, tag="pt", name=f"pt{rb}_{c}")
            if gemm == "fp8":
                # DoubleRow: both 128-dim k-tiles in one matmul
                lw = zt_v[:, :, 128 * rb : 128 * rb + 128]  # [128, 2, 128]
                for b in range(CHUNK // 512):
                    col = CHUNK * c + 512 * b
                    nc.tensor.matmul(
                        pt[:, 512 * b : 512 * b + 512],
                        lhsT=lw,
                        rhs=zt_v[:, :, col : col + 512],
                        perf_mode=mybir.MatmulPerfMode.DoubleRow,
                        start=True,
                        stop=True,
                    )
            else:
                l0 = zt[:, 128 * rb : 128 * rb + 128]
                l1 = zt[:, B + 128 * rb : B + 128 * rb + 128]
                for b in range(CHUNK // 512):
                    col = CHUNK * c + 512 * b
                    nc.tensor.matmul(
                        pt[:, 512 * b : 512 * b + 512],
                        lhsT=l0,
                        rhs=zt[:, col : col + 512],
                        start=True,
                        stop=False,
                    )
                for b in range(CHUNK // 512):
                    col = CHUNK * c + 512 * b
                    nc.tensor.matmul(
                        pt[:, 512 * b : 512 * b + 512],
                        lhsT=l1,
                        rhs=zt[:, B + col : B + col + 512],
                        start=False,
                        stop=True,
                    )

            if c == 0:
                # ship the raw f32 v window (positives + diagonal)
                o = 128 * rb
                wstage = wsp.tile([128, WIN], f32, tag="wstage", name=f"ws{rb}")
                nc.scalar.copy(wstage[:], pt[:, o : o + WIN])
                nc.sync.dma_start(
                    out=wins[:, WIN * rb : WIN * rb + WIN],
                    in_=wstage[:],
                )

            E = Epool.tile([128, CHUNK], bf16, tag="E", name=f"E{rb}_{c}")
            nc.scalar.activation(
                E[:],
                pt[:],
                AF.Exp,
                bias=mbias_sb[:, rb : rb + 1],
                scale=float(invtemp),
                accum_out=stats_sb[:, 8 * rb + c : 8 * rb + c + 1],
            )
            u = upool.tile([128, CHUNK], bf16, tag="u", name=f"u{rb}_{c}")
            if c % 2 == 1:
                # odd chunks: second exp at scale (1-delta) — the finite
                # difference in exponent scale yields sum(s*e^s) on the host
                # (runs on ACT instead of loading DVE with an STT)
                nc.scalar.activation(
                    u[:],
                    pt[:],
                    AF.Exp,
                    bias=mbias_sb[:, RB + rb : RB + rb + 1],
                    scale=float(np.float32(invtemp * (1.0 - fd_delta))),
                    accum_out=stats_sb[:, 8 * rb + 6 + c // 2 : 8 * rb + 7 + c // 2],
                )
            else:
                stt = nc.gpsimd if stt_engine == "pool" else nc.vector
                stt.scalar_tensor_tensor(
                    out=u[:],
                    in0=pt[:],
                    scalar=1.0,
                    in1=E[:],
                    op0=ALU.bypass,
                    op1=ALU.mult,
                    accum_out=stats_sb[:, 8 * rb + 4 + c // 2 : 8 * rb + 5 + c // 2],
                )
            if c == 0:
                # sanitize the diag/positive window with neighbouring
                # (negative) columns so extremes can run on whole tiles
                o = 128 * rb
                nc.scalar.copy(E[:, o : o + WIN], E[:, o + WIN : o + 2 * WIN])
            # inline running min/max update (2x DVE mode on bf16)
            if _state["first_E"] is None and not _state["run_init"]:
                _state["first_E"] = E
            elif _state["run_init"]:
                nc.vector.tensor_tensor(run_mn[:], run_mn[:], E[:], op=ALU.min)
                nc.vector.tensor_tensor(run_mx[:], run_mx[:], E[:], op=ALU.max)
            else:
                first = _state["first_E"]
                nc.vector.tensor_tensor(run_mn[:], first[:], E[:], op=ALU.min)
                nc.vector.tensor_tensor(run_mx[:], first[:], E[:], op=ALU.max)
                _state["first_E"] = None
                _state["run_init"] = True

        _state["first_E"] = None
        _state["run_init"] = False

        nc.sync.dma_start(out=mbias_sb[:], in_=mbias_in)
        for c in range(NCH):
            load_chunk(c)
        for rb in range(RB):
            for c in range(NCH):
                main_block(rb, c)

        G = RB * 8
        nc.vector.tensor_reduce(
            stats_sb[:, G : G + 1], run_mn[:], axis=AX.X, op=ALU.min
        )
        nc.vector.tensor_reduce(
            stats_sb[:, G + 1 : G + 2], run_mx[:], axis=AX.X, op=ALU.max
        )

        nc.sync.dma_start(out=stats, in_=stats_sb[:])

        _state.pop("first_E", None)
        _state.pop("run_init", None)

        _state.pop("acc", None)
        _state.pop("pidx", None)

    nc.compile()
    return nc


# --------------------------------------------------------------------------
# runners
# --------------------------------------------------------------------------

FD_DELTA = 0.01


def _fd_rbs():
    s = os.environ.get("KERNEL_FD_RBS", "0,2,4,6")
    return tuple(int(x) for x in s.split(",") if x != "")


def _get_program(invtemp: float, negc: float):
    stt_engine = os.environ.get("KERNEL_STT_ENGINE", "dve")
    gemm = os.environ.get("KERNEL_GEMM", "fp8")
    fd_rbs = _fd_rbs()
    key = ("prog", float(invtemp), float(negc), stt_engine, gemm, fd_rbs)
    if key not in _state:
        _state[key] = _build_program(invtemp, negc, stt_engine, gemm,
                                     fd_rbs, FD_DELTA)
    return _state[key]


def _run_device_stock(nc, in_maps):
    from concourse.bass_utils import run_bass_kernel_spmd

    res = run_bass_kernel_spmd(nc, in_maps, list(range(NCORES)))
    _state["last_results"] = res
    return res.results


def _make_cached_runner(nc, return_parts=False):
    """Vendored multi-core tail of bass2jax.run_bass_via_pjrt, but keeping the
    jitted callable so repeated invocations (for timing) do not recompile."""
    import jax
    import concourse.mybir as mybir
    from jax.sharding import Mesh, PartitionSpec
    from concourse.bass2jax import (
        _bass_exec_p,
        install_neuronx_cc_hook,
        partition_id_tensor,
    )

    try:
        from jax.experimental.shard_map import shard_map
    except Exception:  # newer jax
        from jax import shard_map  # type: ignore

    install_neuronx_cc_hook()

    partition_name = nc.partition_id_tensor.name if nc.partition_id_tensor else None
    in_names, out_names, out_avals, zero_outs = [], [], [], []
    for alloc in nc.m.functions[0].allocations:
        if not isinstance(alloc, mybir.MemoryLocationSet):
            continue
        name = alloc.memorylocations[0].name
        if alloc.kind == "ExternalInput":
            if name != partition_name:
                in_names.append(name)
        elif alloc.kind == "ExternalOutput":
            out_names.append(name)
            shape = tuple(alloc.tensor_shape)
            dtype = mybir.dt.np(alloc.dtype)
            out_avals.append(jax.core.ShapedArray(shape, dtype))
            zero_outs.append(np.zeros(shape, dtype))
    n_params = len(in_names)
    all_names = in_names + out_names
    if partition_name is not None:
        all_names = all_names + [partition_name]
    donate = tuple(range(n_params, n_params + len(out_names)))

    def _body(*args):
        operands = list(args)
        if partition_name is not None:
            operands.append(partition_id_tensor())
        outs = _bass_exec_p.bind(
            *operands,
            out_avals=tuple(out_avals),
            in_names=tuple(all_names),
            out_names=tuple(out_names),
            lowering_input_output_aliases=(),
            sim_require_finite=True,
            sim_require_nnan=True,
            nc=nc,
        )
        return tuple(outs)

    devices = jax.devices()[:NCORES]
    mesh = Mesh(np.asarray(devices), ("core",))
    n_out = len(out_names)
    sharded = jax.jit(
        shard_map(
            _body,
            mesh=mesh,
            in_specs=(PartitionSpec("core"),) * (n_params + n_out),
            out_specs=(PartitionSpec("core"),) * n_out,
            check_rep=False,
        ),
        donate_argnums=donate,
        keep_unused=True,
    )

    def run(in_maps):
        concat_in = [
            np.concatenate([np.asarray(m[nm]) for m in in_maps], axis=0)
            for nm in in_names
        ]
        concat_zeros = [
            np.zeros((NCORES * z.shape[0], *z.shape[1:]), z.dtype) for z in zero_outs
        ]
        out_arrs = sharded(*concat_in, *concat_zeros)
        return [
            {
                nm: np.asarray(out_arrs[i]).reshape(NCORES, *out_avals[i].shape)[c]
                for i, nm in enumerate(out_names)
            }
            for c in range(NCORES)
        ]

    if return_parts:
        return run, sharded, in_names, out_avals, zero_outs
    return run


def _run_device(nc, in_maps):
    if os.environ.get("KERNEL_FAST_RUNNER"):
        key = ("runner", id(nc))
        if key not in _state:
            _state[key] = _make_cached_runner(nc)
        return _state[key](in_maps)
    return _run_device_stock(nc, in_maps)


# --------------------------------------------------------------------------
# host finish
# --------------------------------------------------------------------------

def _numpy_reference(emb, pos_vals, temperature, pos_row, pos_col):
    """Exact fallback replica of the reference (used only if the positive
    index pattern is not the expected banded structure)."""
    n = emb.shape[0]
    norm = np.sqrt((emb.astype(np.float32) ** 2).sum(1, keepdims=True))
    z = emb / np.maximum(norm, np.float32(1e-12))
    temp = np.float32(np.log1p(np.exp(np.float64(temperature))))
    sim = (z @ z.T) / temp
    sim = sim - sim.max(axis=1, keepdims=True)
    posd = np.zeros((n, n), bool)
    posd[pos_row, pos_col] = True
    negm = ~posd & ~np.eye(n, dtype=bool)
    pos_w = 1.0 - pos_vals
    pos_w = (pos_w - pos_w.min()) / (pos_w.max() - pos_w.min() + np.float32(EPS))
    neg_min = sim[negm].min()
    neg_max = sim[negm].max()
    neg_w = (sim - neg_min) / (neg_max - neg_min + np.float32(EPS)) + 1.0
    logw = np.where(negm, np.log(neg_w), 0.0).astype(np.float32)
    a = (sim + logw).astype(np.float64)
    lse = np.log(np.exp(a).sum(1))
    pl = sim[pos_row, pos_col].astype(np.float64) - lse[pos_row]
    return np.float32(-np.mean(pl * pos_w.astype(np.float64)))


def _host_prepare(emb):
    """Normalize (f64), transpose, cast to the GEMM dtype: zT [D, B].
    Also return the per-row squared norm of the quantized z (the exact sim
    diagonal = per-row max, used as the exp shift)."""
    import ml_dtypes

    gemm = os.environ.get("KERNEL_GEMM", "fp8")
    zdt = ml_dtypes.float8_e4m3 if gemm == "fp8" else ml_dtypes.bfloat16
    e = emb.astype(np.float64)
    nrm = np.sqrt((e * e).sum(1, keepdims=True))
    z = e / np.maximum(nrm, 1e-12)
    zT = np.ascontiguousarray(z.T.astype(np.float32)).astype(zdt)
    zq = zT.astype(np.float32)
    diag_q = (zq * zq).sum(0).astype(np.float32)  # [B]
    return zT, diag_q  # [256, 8192], [8192]


def _make_in_maps(emb, invtemp):
    zT, diag_q = _host_prepare(emb)
    it2 = np.float32(np.float32(invtemp) * np.float32(1.0 - FD_DELTA))
    lam = np.float64(it2) / np.float64(np.float32(invtemp))
    in_maps = []
    for k in range(NCORES):
        dk = np.roll(diag_q, -ROWS * k)[0:ROWS]  # local rows of core k
        mb = np.ascontiguousarray(
            (-np.float32(invtemp) * dk).reshape(RB, 128).T
        ).astype(np.float32)                     # [128, RB]
        mb2 = (mb.astype(np.float64) * lam).astype(np.float32)
        in_maps.append(
            {
                "zt": np.roll(zT, -ROWS * k, axis=1),
                "mbias": np.concatenate([mb, mb2], axis=1),
            }
        )
    return in_maps, diag_q


def kernel(**inputs):
    emb = np.ascontiguousarray(np.asarray(inputs["embeddings"], dtype=np.float32))
    pos_vals = np.asarray(inputs["pos_vals"], dtype=np.float32)
    temperature = np.asarray(inputs["temperature"], dtype=np.float32)
    pos_row = np.asarray(inputs["pos_row"]).astype(np.int64)
    pos_col = np.asarray(inputs["pos_col"]).astype(np.int64)

    rr = np.repeat(np.arange(B, dtype=np.int64), K)
    oo = np.tile(np.arange(1, K + 1, dtype=np.int64), B)
    structured = (
        emb.shape == (B, D)
        and pos_row.shape == (B * K,)
        and np.array_equal(pos_row, rr)
        and np.array_equal(pos_col, (rr + oo) % B)
    )
    if not structured:
        return _numpy_reference(emb, pos_vals, temperature, pos_row, pos_col)

    temp = float(np.log1p(np.exp(np.float64(temperature))))
    invtemp = 1.0 / np.float32(temp)  # f32 to match device immediates
    invtemp = float(np.float32(invtemp))
    c = invtemp  # row max == diagonal == 1/temp (up to bf16 noise; c is a
    # shift constant only, the host uses the exact shipped diagonal)
    negc = float(np.float32(-c))

    nc = _get_program(invtemp, negc)
    in_maps, diag_q = _make_in_maps(emb, invtemp)
    results = _run_device(nc, in_maps)

    # ---- host finish (f64) ----
    it = np.float64(invtemp)
    cc = np.float64(c)

    sumE = np.empty(B)
    sumU = np.empty(B)
    sumE2 = np.empty(B)
    se_even = np.empty(B)
    se_odd = np.empty(B)
    Wv = np.empty((B, WIN))
    emin_glob = np.inf
    emax_glob = -np.inf

    # the device exp shift: s = v*it - m_r with m_r = it * diag_q
    m = diag_q.astype(np.float64) * it

    ridx = np.arange(128)
    G = RB * 8
    for k in range(NCORES):
        stats = results[k]["stats"].astype(np.float64)  # [128, RB*8+2]
        wins = results[k]["wins"].astype(np.float64)    # [128, RB*WIN]
        emin_glob = min(emin_glob, stats[:, G].min())
        emax_glob = max(emax_glob, stats[:, G + 1].max())
        for rb in range(RB):
            g0 = ROWS * k + 128 * rb
            s = stats[:, 8 * rb : 8 * rb + 8]
            W = wins[:, WIN * rb : WIN * rb + WIN]  # [128, 256] raw v
            sumE[g0 : g0 + 128] = s[:, 0:4].sum(1)
            se_even[g0 : g0 + 128] = s[:, 0] + s[:, 2]
            se_odd[g0 : g0 + 128] = s[:, 1] + s[:, 3]
            sumU[g0 : g0 + 128] = s[:, 4] + s[:, 5]
            sumE2[g0 : g0 + 128] = s[:, 6] + s[:, 7]
            Wv[g0 : g0 + 128] = W

    # masked min/max inside window: exclude relative cols r..r+K
    wmin = np.empty(B)
    wmax = np.empty(B)
    for blk in range(B // 128):
        sl = slice(blk * 128, blk * 128 + 128)
        Wm = Wv[sl].copy()
        for o in range(K + 1):
            Wm[ridx, ridx + o] = np.nan
        wmin[sl] = np.nanmin(Wm, axis=1)
        wmax[sl] = np.nanmax(Wm, axis=1)

    # global neg extremes of s = v*it - m_r.  The device exp bias is the
    # exact per-row m_r, so ln(E-extreme) IS the s-extreme.
    neg_min = min(np.log(emin_glob), ((wmin * it) - m).min())
    neg_max = max(np.log(emax_glob), ((wmax * it) - m).max())
    a = 1.0 / (neg_max - neg_min + EPS)
    b = 1.0 - a * neg_min

    # pos/diag (pd) corrections from the raw windows
    rows = np.arange(B)
    r_in_blk = rows % 128
    pd_idx = r_in_blk[:, None] + np.arange(K + 1)[None, :]   # [B, 9] window cols
    v_pd = Wv[rows[:, None], pd_idx]                         # raw v at diag+pos
    s_pd = v_pd * it - m[:, None]                            # s = v*it - m_r
    E_pd = np.exp(s_pd)
    sum_pd_E = E_pd.sum(1)
    sum_pd_sE = (s_pd * E_pd).sum(1)

    # sum sE over all cols: exact (it*sumU - m*sumE) on even chunks (STT),
    # finite difference (se_odd - sumE2)/delta_eff on odd chunks
    it2 = np.float32(np.float32(invtemp) * np.float32(1.0 - FD_DELTA))
    delta_eff = 1.0 - np.float64(it2) / np.float64(np.float32(invtemp))
    A_all = (it * sumU - m * se_even) + (se_odd - sumE2) / delta_eff
    A_neg = A_all - sum_pd_sE
    B_neg = sumE - sum_pd_E

    Sw = a * A_neg + b * B_neg + sum_pd_E
    log_sw = np.log(Sw)

    # positive log-probs: pos o (o=1..K) of row r is window col r_in_blk+o
    v_pos = v_pd[:, 1:]                      # [B, K]
    pos_log = v_pos * it - m[:, None] - log_sw[:, None]

    pos_w = 1.0 - pos_vals.astype(np.float64)
    pos_w = (pos_w - pos_w.min()) / (pos_w.max() - pos_w.min() + EPS)
    loss = -np.mean(pos_log.reshape(-1) * pos_w)
    return np.float32(loss)


# revision 61
# speedup vs baseline: 1.1674x; 1.0044x over previous
"""Trainium2 Bass kernel for nn_ContrastiveLoss_82300163326281.

Strategy (8 NeuronCores, SPMD, no collectives):
  - Host: L2-normalize the embeddings (f64), transpose to zT [D, B], cast to
    fp8e4m3.  Core k receives roll(zT, -1024k, axis=1) so every core runs
    the same program over its local rows 0..1023 (row reductions are
    permutation invariant; the diagonal/positive window sits at a fixed
    local offset).  The host also sends the exact per-row shift
    m_r = it*||q(z_r)||^2 (the sim diagonal = row max) as the exp bias.
  - Device, per core (1024 local rows of sim), per 128-row x 2048-col block:
      PE   : fp8 DoubleRow matmul (256-contraction per instr) -> PSUM v f32
      ACT  : E = exp(v*it - m_r) -> bf16 SBUF, accum -> per-chunk rowsum(E);
             on odd chunks additionally E2 = exp((1-d)*(v*it - m_r)) with
             accum (finite difference in exponent scale gives sum(s*e^s)
             without DVE work)
      DVE  : on even chunks scalar_tensor_tensor v*E, accum -> rowsum(v*E);
             running elementwise min/max of E tiles (tensor_tensor, 2x bf16
             mode) into global [128, 2048] extreme tiles -- since the shift
             is the exact row max, ln(E-extreme) IS the s-extreme, so
             row/column mixing loses nothing
      ACT  : chunk 0: ship raw f32 v window [128,256] (diag+positives) and
             overwrite it in E with neighbouring negative columns so the
             extremes can run on whole tiles
  - Host finish (f64): global neg_min/neg_max from ln of device E-extremes
    merged with an exact window scan; affine decomposition of the
    'inverse_sim' weights  w = a*s + b  so that
      sum_j w_j e^{s_j} = a*sum(s e^s) + b*sum(e^s) (+ window corrections),
    where sum(s e^s) combines the exact STT part (even chunks) and the
    finite-difference part (odd chunks); positive log-probs from the
    shipped windows, weighted mean.

Self-contained: hardcodes shapes; falls back to a pure-numpy replica of the
reference if the positive-index structure is not the expected banded pattern.
"""

import os
import sys

import numpy as np

sys.path.insert(0, "/opt/trn_rl_repo")

B = 8192
D = 256
K = 8
NCORES = 8
ROWS = B // NCORES          # 1024 rows per core
RB = ROWS // 128            # 8 row blocks per core
CHUNK = 2048
NCH = B // CHUNK            # 4 column chunks
WIN = 256                   # diagonal window width (>= 128 + K + 1)
EPS = 1e-8

_state = {}


# --------------------------------------------------------------------------
# device program
# --------------------------------------------------------------------------

def _build_program(invtemp: float, negc: float, stt_engine: str = "dve",
                  gemm: str = "fp8", fd_delta: float = 0.01):
    from contextlib import ExitStack

    import concourse.bass as bass  # noqa: F401
    import concourse.mybir as mybir
    from concourse import bacc, tile

    f32 = mybir.dt.float32
    bf16 = mybir.dt.bfloat16
    zdt = mybir.dt.float8e4 if gemm == "fp8" else mybir.dt.bfloat16
    AF = mybir.ActivationFunctionType
    ALU = mybir.AluOpType
    AX = mybir.AxisListType

    nc = bacc.Bacc(
        "TRN2",
        target_bir_lowering=False,
        debug=False,
        num_devices=NCORES,
    )
    zt_in = nc.dram_tensor("zt", [2 * 128, B], zdt, kind="ExternalInput").ap()
    # cols 0..RB-1: -it*m_r ; cols RB..2RB-1: (1-delta) * that
    mbias_in = nc.dram_tensor("mbias", [128, 2 * RB], f32, kind="ExternalInput").ap()
    # per rb: [0..3]=sumE per chunk, [4..5]=sumU (chunks 0,2),
    # [6..7]=sumE2 (chunks 1,3); cols 64,65: global E min/max
    stats = nc.dram_tensor("stats", [128, RB * 8 + 2], f32, kind="ExternalOutput").ap()
    wins = nc.dram_tensor("wins", [128, RB * WIN], f32, kind="ExternalOutput").ap()

    with tile.TileContext(nc) as tc, ExitStack() as ctx:
        ztp = ctx.enter_context(tc.tile_pool(name="ztp", bufs=1))
        # zt[:, 0:B] = dims 0..127, zt[:, B:2B] = dims 128..255
        zt = ztp.tile([128, 2 * B], zdt, tag="zt", name="zt")

        psum = ctx.enter_context(tc.tile_pool(name="psum", bufs=2, space="PSUM"))
        Epool = ctx.enter_context(tc.tile_pool(name="Epool", bufs=3))
        runp = ctx.enter_context(tc.tile_pool(name="runp", bufs=1))
        upool = ctx.enter_context(tc.tile_pool(name="upool", bufs=2))
        outp = ctx.enter_context(tc.tile_pool(name="outp", bufs=1))
        wsp = ctx.enter_context(tc.tile_pool(name="wsp", bufs=2))

        stats_sb = outp.tile([128, RB * 8 + 2], f32, tag="stats_sb", name="stats_sb")
        nc.gpsimd.memset(stats_sb[:], 0.0)

        # per-row exp biases (host-computed exact diagonal, and scaled copy)
        mbias_sb = outp.tile([128, 2 * RB], f32, tag="mbias_sb", name="mbias_sb")

        # global running elementwise min/max of E across blocks/chunks.  The
        # exp bias is the exact per-row max m_r (the diagonal), so ln of a
        # global E-extreme IS the global extreme of s = v*it - m_r: row/col
        # mixing across blocks loses nothing.
        run_mn = runp.tile([128, CHUNK], bf16, tag="run_mn", name="run_mn")
        run_mx = runp.tile([128, CHUNK], bf16, tag="run_mx", name="run_mx")

        zt_r = zt_in.rearrange("(t p) c -> p t c", p=128)  # [128, 2, B]

        def load_chunk(c):
            for t in range(2):
                nc.sync.dma_start(
                    out=zt[:, t * B + c * CHUNK : t * B + (c + 1) * CHUNK],
                    in_=zt_r[:, t, c * CHUNK : (c + 1) * CHUNK],
                )

        zt_v = zt[:].rearrange("p (t c) -> p t c", t=2)  # [128, 2, B]

        def main_block(rb, c):  # noqa: C901
            pt = psum.tile([128, CHUNK], f32, tag="pt", name=f"pt{rb}_{c}")
            if gemm == "fp8":
                # DoubleRow: both 128-dim k-tiles in one matmul
                lw = zt_v[:, :, 128 * rb : 128 * rb + 128]  # [128, 2, 128]
                for b in range(CHUNK // 512):
                    col = CHUNK * c + 512 * b
                    nc.tensor.matmul(
                        pt[:, 512 * b : 512 * b + 512],
                        lhsT=lw,
                        rhs=zt_v[:, :, col : col + 512],
                        perf_mode=mybir.MatmulPerfMode.DoubleRow,
                        start=True,
                        stop=True,
                    )
            else:
                l0 = zt[:, 128 * rb : 128 * rb + 128]
                l1 = zt[:, B + 128 * rb : B + 128 * rb + 128]
                for b in range(CHUNK // 512):
                    col = CHUNK * c + 512 * b
                    nc.tensor.matmul(
                        pt[:, 512 * b : 512 * b + 512],
                        lhsT=l0,
                        rhs=zt[:, col : col + 512],
                        start=True,
                        stop=False,
                    )
                for b in range(CHUNK // 512):
                    col = CHUNK * c + 512 * b
                    nc.tensor.matmul(
                        pt[:, 512 * b : 512 * b + 512],
                        lhsT=l1,
                        rhs=zt[:, B + col : B + col + 512],
                        start=False,
                        stop=True,
                    )

            if c == 0:
                # ship the raw f32 v window (positives + diagonal)
                o = 128 * rb
                wstage = wsp.tile([128, WIN], f32, tag="wstage", name=f"ws{rb}")
                nc.scalar.copy(wstage[:], pt[:, o : o + WIN])
                nc.sync.dma_start(
                    out=wins[:, WIN * rb : WIN * rb + WIN],
                    in_=wstage[:],
                )

            E = Epool.tile([128, CHUNK], bf16, tag="E", name=f"E{rb}_{c}")
            nc.scalar.activation(
                E[:],
                pt[:],
                AF.Exp,
                bias=mbias_sb[:, rb : rb + 1],
                scale=float(invtemp),
                accum_out=stats_sb[:, 8 * rb + c : 8 * rb + c + 1],
            )
            u = upool.tile([128, CHUNK], bf16, tag="u", name=f"u{rb}_{c}")
            if c % 2 == 1:
                # odd chunks: second exp at scale (1-delta) — the finite
                # difference in exponent scale yields sum(s*e^s) on the host
                # (runs on ACT instead of loading DVE with an STT)
                nc.scalar.activation(
                    u[:],
                    pt[:],
                    AF.Exp,
                    bias=mbias_sb[:, RB + rb : RB + rb + 1],
                    scale=float(np.float32(invtemp * (1.0 - fd_delta))),
                    accum_out=stats_sb[:, 8 * rb + 6 + c // 2 : 8 * rb + 7 + c // 2],
                )
            else:
                stt = nc.gpsimd if stt_engine == "pool" else nc.vector
                stt.scalar_tensor_tensor(
                    out=u[:],
                    in0=pt[:],
                    scalar=1.0,
                    in1=E[:],
                    op0=ALU.bypass,
                    op1=ALU.mult,
                    accum_out=stats_sb[:, 8 * rb + 4 + c // 2 : 8 * rb + 5 + c // 2],
                )
            if c == 0:
                # sanitize the diag/positive window with neighbouring
                # (negative) columns so extremes can run on whole tiles
                o = 128 * rb
                nc.scalar.copy(E[:, o : o + WIN], E[:, o + WIN : o + 2 * WIN])
            # inline running min/max update (2x DVE mode on bf16)
            if _state["first_E"] is None and not _state["run_init"]:
                _state["first_E"] = E
            elif _state["run_init"]:
                nc.vector.tensor_tensor(run_mn[:], run_mn[:], E[:], op=ALU.min)
                nc.vector.tensor_tensor(run_mx[:], run_mx[:], E[:], op=ALU.max)
            else:
                first = _state["first_E"]
                nc.vector.tensor_tensor(run_mn[:], first[:], E[:], op=ALU.min)
                nc.vector.tensor_tensor(run_mx[:], first[:], E[:], op=ALU.max)
                _state["first_E"] = None
                _state["run_init"] = True

        _state["first_E"] = None
        _state["run_init"] = False

        nc.sync.dma_start(out=mbias_sb[:], in_=mbias_in)
        for c in range(NCH):
            load_chunk(c)
        for rb in range(RB):
            for c in range(NCH):
                main_block(rb, c)

        G = RB * 8
        nc.vector.tensor_reduce(
            stats_sb[:, G : G + 1], run_mn[:], axis=AX.X, op=ALU.min
        )
        nc.vector.tensor_reduce(
            stats_sb[:, G + 1 : G + 2], run_mx[:], axis=AX.X, op=ALU.max
        )

        nc.sync.dma_start(out=stats, in_=stats_sb[:])

        _state.pop("first_E", None)
        _state.pop("run_init", None)

        _state.pop("acc", None)
        _state.pop("pidx", None)

    nc.compile()
    return nc


# --------------------------------------------------------------------------
# runners
# --------------------------------------------------------------------------

FD_DELTA = 0.01


def _get_program(invtemp: float, negc: float):
    stt_engine = os.environ.get("KERNEL_STT_ENGINE", "dve")
    gemm = os.environ.get("KERNEL_GEMM", "fp8")
    key = ("prog", float(invtemp), float(negc), stt_engine, gemm)
    if key not in _state:
        _state[key] = _build_program(invtemp, negc, stt_engine, gemm, FD_DELTA)
    return _state[key]


def _run_device_stock(nc, in_maps):
    from concourse.bass_utils import run_bass_kernel_spmd

    res = run_bass_kernel_spmd(nc, in_maps, list(range(NCORES)))
    _state["last_results"] = res
    return res.results


def _make_cached_runner(nc, return_parts=False):
    """Vendored multi-core tail of bass2jax.run_bass_via_pjrt, but keeping the
    jitted callable so repeated invocations (for timing) do not recompile."""
    import jax
    import concourse.mybir as mybir
    from jax.sharding import Mesh, PartitionSpec
    from concourse.bass2jax import (
        _bass_exec_p,
        install_neuronx_cc_hook,
        partition_id_tensor,
    )

    try:
        from jax.experimental.shard_map import shard_map
    except Exception:  # newer jax
        from jax import shard_map  # type: ignore

    install_neuronx_cc_hook()

    partition_name = nc.partition_id_tensor.name if nc.partition_id_tensor else None
    in_names, out_names, out_avals, zero_outs = [], [], [], []
    for alloc in nc.m.functions[0].allocations:
        if not isinstance(alloc, mybir.MemoryLocationSet):
            continue
        name = alloc.memorylocations[0].name
        if alloc.kind == "ExternalInput":
            if name != partition_name:
                in_names.append(name)
        elif alloc.kind == "ExternalOutput":
            out_names.append(name)
            shape = tuple(alloc.tensor_shape)
            dtype = mybir.dt.np(alloc.dtype)
            out_avals.append(jax.core.ShapedArray(shape, dtype))
            zero_outs.append(np.zeros(shape, dtype))
    n_params = len(in_names)
    all_names = in_names + out_names
    if partition_name is not None:
        all_names = all_names + [partition_name]
    donate = tuple(range(n_params, n_params + len(out_names)))

    def _body(*args):
        operands = list(args)
        if partition_name is not None:
            operands.append(partition_id_tensor())
        outs = _bass_exec_p.bind(
            *operands,
            out_avals=tuple(out_avals),
            in_names=tuple(all_names),
            out_names=tuple(out_names),
            lowering_input_output_aliases=(),
            sim_require_finite=True,
            sim_require_nnan=True,
            nc=nc,
        )
        return tuple(outs)

    devices = jax.devices()[:NCORES]
    mesh = Mesh(np.asarray(devices), ("core",))
    n_out = len(out_names)
    sharded = jax.jit(
        shard_map(
            _body,
            mesh=mesh,
            in_specs=(PartitionSpec("core"),) * (n_params + n_out),
            out_specs=(PartitionSpec("core"),) * n_out,
            check_rep=False,
        ),
        donate_argnums=donate,
        keep_unused=True,
    )

    def run(in_maps):
        concat_in = [
            np.concatenate([np.asarray(m[nm]) for m in in_maps], axis=0)
            for nm in in_names
        ]
        concat_zeros = [
            np.zeros((NCORES * z.shape[0], *z.shape[1:]), z.dtype) for z in zero_outs
        ]
        out_arrs = sharded(*concat_in, *concat_zeros)
        return [
            {
                nm: np.asarray(out_arrs[i]).reshape(NCORES, *out_avals[i].shape)[c]
                for i, nm in enumerate(out_names)
            }
            for c in range(NCORES)
        ]

    if return_parts:
        return run, sharded, in_names, out_avals, zero_outs
    return run


def _run_device(nc, in_maps):
    if os.environ.get("KERNEL_FAST_RUNNER"):
        key = ("runner", id(nc))
        if key not in _state:
            _state[key] = _make_cached_runner(nc)
        return _state[key](in_maps)
    return _run_device_stock(nc, in_maps)


# --------------------------------------------------------------------------
# host finish
# --------------------------------------------------------------------------

def _numpy_reference(emb, pos_vals, temperature, pos_row, pos_col):
    """Exact fallback replica of the reference (used only if the positive
    index pattern is not the expected banded structure)."""
    n = emb.shape[0]
    norm = np.sqrt((emb.astype(np.float32) ** 2).sum(1, keepdims=True))
    z = emb / np.maximum(norm, np.float32(1e-12))
    temp = np.float32(np.log1p(np.exp(np.float64(temperature))))
    sim = (z @ z.T) / temp
    sim = sim - sim.max(axis=1, keepdims=True)
    posd = np.zeros((n, n), bool)
    posd[pos_row, pos_col] = True
    negm = ~posd & ~np.eye(n, dtype=bool)
    pos_w = 1.0 - pos_vals
    pos_w = (pos_w - pos_w.min()) / (pos_w.max() - pos_w.min() + np.float32(EPS))
    neg_min = sim[negm].min()
    neg_max = sim[negm].max()
    neg_w = (sim - neg_min) / (neg_max - neg_min + np.float32(EPS)) + 1.0
    logw = np.where(negm, np.log(neg_w), 0.0).astype(np.float32)
    a = (sim + logw).astype(np.float64)
    lse = np.log(np.exp(a).sum(1))
    pl = sim[pos_row, pos_col].astype(np.float64) - lse[pos_row]
    return np.float32(-np.mean(pl * pos_w.astype(np.float64)))


def _host_prepare(emb):
    """Normalize (f64), transpose, cast to the GEMM dtype: zT [D, B].
    Also return the per-row squared norm of the quantized z (the exact sim
    diagonal = per-row max, used as the exp shift)."""
    import ml_dtypes

    gemm = os.environ.get("KERNEL_GEMM", "fp8")
    zdt = ml_dtypes.float8_e4m3 if gemm == "fp8" else ml_dtypes.bfloat16
    e = emb.astype(np.float64)
    nrm = np.sqrt((e * e).sum(1, keepdims=True))
    z = e / np.maximum(nrm, 1e-12)
    zT = np.ascontiguousarray(z.T.astype(np.float32)).astype(zdt)
    zq = zT.astype(np.float32)
    diag_q = (zq * zq).sum(0).astype(np.float32)  # [B]
    return zT, diag_q  # [256, 8192], [8192]


def _make_in_maps(emb, invtemp):
    zT, diag_q = _host_prepare(emb)
    it2 = np.float32(np.float32(invtemp) * np.float32(1.0 - FD_DELTA))
    lam = np.float64(it2) / np.float64(np.float32(invtemp))
    in_maps = []
    for k in range(NCORES):
        dk = np.roll(diag_q, -ROWS * k)[0:ROWS]  # local rows of core k
        mb = np.ascontiguousarray(
            (-np.float32(invtemp) * dk).reshape(RB, 128).T
        ).astype(np.float32)                     # [128, RB]
        mb2 = (mb.astype(np.float64) * lam).astype(np.float32)
        in_maps.append(
            {
                "zt": np.roll(zT, -ROWS * k, axis=1),
                "mbias": np.concatenate([mb, mb2], axis=1),
            }
        )
    return in_maps, diag_q


def kernel(**inputs):
    emb = np.ascontiguousarray(np.asarray(inputs["embeddings"], dtype=np.float32))
    pos_vals = np.asarray(inputs["pos_vals"], dtype=np.float32)
    temperature = np.asarray(inputs["temperature"], dtype=np.float32)
    pos_row = np.asarray(inputs["pos_row"]).astype(np.int64)
    pos_col = np.asarray(inputs["pos_col"]).astype(np.int64)

    rr = np.repeat(np.arange(B, dtype=np.int64), K)
    oo = np.tile(np.arange(1, K + 1, dtype=np.int64), B)
    structured = (
        emb.shape == (B, D)
        and pos_row.shape == (B * K,)
        and np.array_equal(pos_row, rr)
        and np.array_equal(pos_col, (rr + oo) % B)
    )
    if not structured:
        return _numpy_reference(emb, pos_vals, temperature, pos_row, pos_col)

    temp = float(np.log1p(np.exp(np.float64(temperature))))
    invtemp = 1.0 / np.float32(temp)  # f32 to match device immediates
    invtemp = float(np.float32(invtemp))
    c = invtemp  # row max == diagonal == 1/temp (up to bf16 noise; c is a
    # shift constant only, the host uses the exact shipped diagonal)
    negc = float(np.float32(-c))

    nc = _get_program(invtemp, negc)
    in_maps, diag_q = _make_in_maps(emb, invtemp)
    results = _run_device(nc, in_maps)

    # ---- host finish (f64) ----
    it = np.float64(invtemp)
    cc = np.float64(c)

    sumE = np.empty(B)
    sumU = np.empty(B)
    sumE2 = np.empty(B)
    se_even = np.empty(B)
    se_odd = np.empty(B)
    Wv = np.empty((B, WIN))
    emin_glob = np.inf
    emax_glob = -np.inf

    # the device exp shift: s = v*it - m_r with m_r = it * diag_q
    m = diag_q.astype(np.float64) * it

    ridx = np.arange(128)
    G = RB * 8
    for k in range(NCORES):
        stats = results[k]["stats"].astype(np.float64)  # [128, RB*8+2]
        wins = results[k]["wins"].astype(np.float64)    # [128, RB*WIN]
        emin_glob = min(emin_glob, stats[:, G].min())
        emax_glob = max(emax_glob, stats[:, G + 1].max())
        for rb in range(RB):
            g0 = ROWS * k + 128 * rb
            s = stats[:, 8 * rb : 8 * rb + 8]
            W = wins[:, WIN * rb : WIN * rb + WIN]  # [128, 256] raw v
            sumE[g0 : g0 + 128] = s[:, 0:4].sum(1)
            se_even[g0 : g0 + 128] = s[:, 0] + s[:, 2]
            se_odd[g0 : g0 + 128] = s[:, 1] + s[:, 3]
            sumU[g0 : g0 + 128] = s[:, 4] + s[:, 5]
            sumE2[g0 : g0 + 128] = s[:, 6] + s[:, 7]
            Wv[g0 : g0 + 128] = W

    # masked min/max inside window: exclude relative cols r..r+K
    wmin = np.empty(B)
    wmax = np.empty(B)
    for blk in range(B // 128):
        sl = slice(blk * 128, blk * 128 + 128)
        Wm = Wv[sl].copy()
        for o in range(K + 1):
            Wm[ridx, ridx + o] = np.nan
        wmin[sl] = np.nanmin(Wm, axis=1)
        wmax[sl] = np.nanmax(Wm, axis=1)

    # global neg extremes of s = v*it - m_r.  The device exp bias is the
    # exact per-row m_r, so ln(E-extreme) IS the s-extreme.
    neg_min = min(np.log(emin_glob), ((wmin * it) - m).min())
    neg_max = max(np.log(emax_glob), ((wmax * it) - m).max())
    a = 1.0 / (neg_max - neg_min + EPS)
    b = 1.0 - a * neg_min

    # pos/diag (pd) corrections from the raw windows
    rows = np.arange(B)
    r_in_blk = rows % 128
    pd_idx = r_in_blk[:, None] + np.arange(K + 1)[None, :]   # [B, 9] window cols
    v_pd = Wv[rows[:, None], pd_idx]                         # raw v at diag+pos
    s_pd = v_pd * it - m[:, None]                            # s = v*it - m_r
    E_pd = np.exp(s_pd)
    sum_pd_E = E_pd.sum(1)
    sum_pd_sE = (s_pd * E_pd).sum(1)

    # sum sE over all cols: exact (it*sumU - m*sumE) on even chunks (STT),
    # finite difference (se_odd - sumE2)/delta_eff on odd chunks
    it2 = np.float32(np.float32(invtemp) * np.float32(1.0 - FD_DELTA))
    delta_eff = 1.0 - np.float64(it2) / np.float64(np.float32(invtemp))
    A_all = (it * sumU - m * se_even) + (se_odd - sumE2) / delta_eff
    A_neg = A_all - sum_pd_sE
    B_neg = sumE - sum_pd_E

    Sw = a * A_neg + b * B_neg + sum_pd_E
    log_sw = np.log(Sw)

    # positive log-probs: pos o (o=1..K) of row r is window col r_in_blk+o
    v_pos = v_pd[:, 1:]                      # [B, K]
    pos_log = v_pos * it - m[:, None] - log_sw[:, None]

    pos_w = 1.0 - pos_vals.astype(np.float64)
    pos_w = (pos_w - pos_w.min()) / (pos_w.max() - pos_w.min() + EPS)
    loss = -np.mean(pos_log.reshape(-1) * pos_w)
    return np.float32(loss)
